# revision 41
# baseline (speedup 1.0000x reference)
"""AttnBlock++ (GroupNorm + 1x1-conv QKV + full spatial self-attention + proj + residual)
for Trainium2, data-parallel over batch across 8 NeuronCores.

Reference computation (per image, C=512 channels, N=HW=1024 pixels):
  h   = GroupNorm32(x) * gamma + beta
  q,k,v = Wq h + bq, Wk h + bk, Wv h + bv          (1x1 convs = channel matmuls)
  S   = q^T k / sqrt(C);  P = softmax_rows(S);  a = v P^T
  out = x + Wp a + bp

Kernel strategy (per core, 4 images):
  - channels live on partitions in 4 chunks of 128; pixels on the free dim.
  - S is computed TRANSPOSED (key index m on partitions) so the AV contraction
    (over m) is a natural matmul; softmax runs without max subtraction (exp of
    unit-variance scores fits fp32 comfortably); denominators are accumulated
    with a ones-vector matmul and divided out only at the very end (softmax
    normalization commutes with the channel projections).
  - matmul operands are fp16 (PSUM accumulation stays fp32): ~3e-4 max rel
    error per matmul at 4x the fp32 matmul throughput.
  - GroupNorm cross-partition group sums use tiny fp32 matmuls against a
    one-hot group-indicator matrix; rstd is a Quake-style rsqrt (bit-trick
    seed + 2 Newton steps) on the vector engine so the scalar engine never
    leaves the exp activation-table set (a Sqrt/Ln would force a table
    reload per image, ~1.3us each).
  - per-image prep (x load + GroupNorm) of image i+1 is emitted in the middle
    of image i's attention so its DMA/DVE work hides under matmuls.
  - all weight transposes / layout shuffles / bias folds (bv folds into an
    effective bp since softmax rows sum to 1) happen on the host in numpy.
"""

import numpy as np

import concourse.bacc as bacc
import concourse.tile as tile
import concourse.mybir as mybir
from concourse.bass import ts
from concourse.bass_utils import run_bass_kernel_spmd

F32 = mybir.dt.float32
F16 = mybir.dt.float16
AF = mybir.ActivationFunctionType
OP = mybir.AluOpType

B, C, H, W = 32, 512, 32, 32
HW = H * W                    # 1024 pixels
NCORES = 8
IPC = B // NCORES             # images per core
P = 128                       # partitions
CC = C // P                   # 4 channel chunks
MC = HW // P                  # 8 pixel chunks (key index)
NH = HW // 512                # 2 free-dim halves of the pixel axis
NGROUPS = 32
GSIZE = C // NGROUPS          # 16 channels per group
EPS = 1e-5
ISC = float(C) ** -0.5

_CACHE = {}


class _Ctx:
    pass


def _build():
    if "nc" in _CACHE:
        return _CACHE["nc"]
    nc = bacc.Bacc("TRN2", target_bir_lowering=False, debug=False, num_devices=NCORES)

    c = _Ctx()
    c.nc = nc
    c.x_d = nc.dram_tensor("x", (IPC, P, CC, HW), F32, kind="ExternalInput")
    c.wq_d = nc.dram_tensor("wqt", (P, CC, C), F16, kind="ExternalInput")
    c.wk_d = nc.dram_tensor("wkt", (P, CC, C), F16, kind="ExternalInput")
    c.wv_d = nc.dram_tensor("wvt", (P, CC, C), F16, kind="ExternalInput")
    c.wp_d = nc.dram_tensor("wpt", (P, CC, C), F16, kind="ExternalInput")
    c.bq_d = nc.dram_tensor("bq", (P, CC), F32, kind="ExternalInput")
    c.bk_d = nc.dram_tensor("bk", (P, CC), F32, kind="ExternalInput")
    c.bpe_d = nc.dram_tensor("bpe", (P, CC), F32, kind="ExternalInput")
    c.gam_d = nc.dram_tensor("gam", (P, CC), F32, kind="ExternalInput")
    c.bet_d = nc.dram_tensor("bet", (P, CC), F32, kind="ExternalInput")
    c.g_d = nc.dram_tensor("gmat", (P, CC, NGROUPS), F32, kind="ExternalInput")
    c.b_d = nc.dram_tensor("bmat", (NGROUPS, CC, P), F32, kind="ExternalInput")
    c.out_d = nc.dram_tensor("out", (IPC, P, CC, HW), F32, kind="ExternalOutput")

    with tile.TileContext(nc) as tc:
        with (
            tc.tile_pool(name="consts", bufs=1) as cp,
            tc.tile_pool(name="xp", bufs=2) as xp,
            tc.tile_pool(name="hp", bufs=2) as hp,
            tc.tile_pool(name="qp", bufs=2) as qp,
            tc.tile_pool(name="kp", bufs=2) as kp,
            tc.tile_pool(name="vp", bufs=2) as vp,
            tc.tile_pool(name="ep", bufs=2) as ep,
            tc.tile_pool(name="up", bufs=2) as up,
            tc.tile_pool(name="rp", bufs=2) as rp,
            tc.tile_pool(name="smalls", bufs=4) as sp,
            tc.tile_pool(name="dtree", bufs=2) as dp,
            tc.tile_pool(name="ps", bufs=4, space="PSUM") as ps,
            tc.tile_pool(name="psd", bufs=2, space="PSUM") as psd,
            tc.tile_pool(name="pss", bufs=2, space="PSUM") as pss,
        ):
            c.tc, c.cp, c.xp, c.hp, c.qp, c.kp, c.vp, c.ep, c.up = \
                tc, cp, xp, hp, qp, kp, vp, ep, up
            c.rp, c.sp, c.dp, c.ps, c.psd, c.pss = rp, sp, dp, ps, psd, pss

            c.wq_s = cp.tile([P, CC, C], F16, tag="wq")
            c.wk_s = cp.tile([P, CC, C], F16, tag="wk")
            c.wv_s = cp.tile([P, CC, C], F16, tag="wv")
            c.wp_s = cp.tile([P, CC, C], F16, tag="wp")
            c.bq_s = cp.tile([P, CC], F32, tag="bq")
            c.bk_s = cp.tile([P, CC], F32, tag="bk")
            c.bpe_s = cp.tile([P, CC], F32, tag="bpe")
            c.gam_s = cp.tile([P, CC], F32, tag="gam")
            c.bet_s = cp.tile([P, CC], F32, tag="bet")
            c.g_sb = cp.tile([P, CC, NGROUPS], F32, tag="gmat")
            c.b_sb = cp.tile([NGROUPS, CC, P], F32, tag="bmat")
            c.ones16 = cp.tile([P, 1], F16, tag="ones16")
            c.onesrow = cp.tile([1, P], F16, tag="onesrow")
            # constant exp shift: exp(s - 6*ln2) = exp(s)/64. Cancels exactly in
            # the softmax ratio but keeps the fp16 denominator row (a sum of
            # 1024 exps) far from fp16 overflow.
            c.eshift = cp.tile([P, 1], F32, tag="eshift")

            # small consts first so image 0's GroupNorm isn't queued behind
            # the 2 MB of projection weights
            for dst, src in (
                (c.g_sb, c.g_d), (c.b_sb, c.b_d), (c.gam_s, c.gam_d),
                (c.bet_s, c.bet_d), (c.bq_s, c.bq_d), (c.bk_s, c.bk_d),
                (c.bpe_s, c.bpe_d),
            ):
                nc.gpsimd.dma_start(out=dst[:], in_=src.ap())
            nc.vector.memset(c.ones16[:], 1.0)
            nc.vector.memset(c.onesrow[:], 1.0)
            nc.vector.memset(c.eshift[:], -6.0 * float(np.log(2.0)))

            # PE warmup: image 0's GroupNorm leaves the PE idle for ~10us at
            # kernel start; junk matmuls in that window ramp the clock gate to
            # full speed before the first real matmul arrives.
            wk_t = c.cp.tile([P, 512], F16, tag="warm")
            nc.vector.memset(wk_t[:], 0.0)
            for i in range(16):
                pw = c.ps.tile([P, 512], F32, tag="mm")
                nc.tensor.matmul(pw[:], wk_t[:, :P], wk_t[:], start=True, stop=True)

            pending = _prep(c, 0)
            for dst, src in (
                (c.wq_s, c.wq_d), (c.wk_s, c.wk_d), (c.wv_s, c.wv_d),
                (c.wp_s, c.wp_d),
            ):
                nc.gpsimd.dma_start(out=dst[:], in_=src.ap())
            for img in range(IPC):
                nxt = _attn(c, img, pending,
                            prep_next=(lambda: _prep(c, img + 1))
                            if img + 1 < IPC else None)
                pending = nxt

    nc.compile()
    _CACHE["nc"] = nc
    return nc


def _prep(c, img):
    """Emit x load + GroupNorm for one image; returns (x_s, h_s)."""
    nc = c.nc
    x_s = c.xp.tile([P, CC, HW], F32, tag="x")
    # per-partition [sum, sumsq] per half-chunk (halves pipeline the DMA)
    scol = c.sp.tile([P, CC, 2, 2], F32, tag="scol")
    junk = c.sp.tile([P, 512], F16, tag="junk")     # discarded Square output
    for ci in range(CC):
        nc.sync.dma_start(out=x_s[:, ci, :], in_=c.x_d.ap()[img, :, ci, :])
        for hf in range(2):
            sl = x_s[:, ci, ts(hf, 512)]
            nc.vector.reduce_sum(out=scol[:, ci, hf, 0:1], in_=sl,
                                 axis=mybir.AxisListType.X)
            nc.scalar.activation(out=junk[:], in_=sl, func=AF.Square,
                                 accum_out=scol[:, ci, hf, 1:2])
    gs_p = c.pss.tile([P, 2], F32, tag="small")     # group [sum, sumsq]
    for i, (ci, hf) in enumerate([(ci, hf) for ci in range(CC) for hf in range(2)]):
        nc.tensor.matmul(gs_p[:NGROUPS, :], c.g_sb[:, ci, :], scol[:, ci, hf, :],
                         start=(i == 0), stop=(i == 2 * CC - 1))
    inv_n = 1.0 / float(GSIZE * HW)
    mv = c.sp.tile([NGROUPS, 2], F32, tag="mv")     # [mean, E(x^2)]
    nc.vector.tensor_scalar_mul(mv[:], gs_p[:NGROUPS, :], inv_n)
    stats = c.sp.tile([NGROUPS, 2], F32, tag="stats")  # [mean, rstd]
    nc.vector.tensor_copy(out=stats[:, 0:1], in_=mv[:, 0:1])
    var = c.sp.tile([NGROUPS, 1], F32, tag="var")
    nc.vector.tensor_mul(var[:], mv[:, 0:1], mv[:, 0:1])
    nc.vector.tensor_sub(var[:], mv[:, 1:2], var[:])
    # rstd = 1/sqrt(var+eps) via Quake seed + 2 Newton steps, all on DVE.
    # (a Sqrt/Ln on ACT would force an activation-table reload per image; the
    # exp set stays resident this way)
    I32 = mybir.dt.int32
    nc.vector.tensor_scalar_add(var[:], var[:], EPS)
    y = c.sp.tile([NGROUPS, 1], F32, tag="rsq_y")
    u = c.sp.tile([NGROUPS, 1], F32, tag="rsq_u")
    nc.vector.tensor_scalar(out=y[:].bitcast(I32), in0=var[:].bitcast(I32),
                            scalar1=1, scalar2=None,
                            op0=OP.logical_shift_right)
    nc.vector.tensor_scalar(out=y[:].bitcast(I32), in0=y[:].bitcast(I32),
                            scalar1=-1, scalar2=0x5F3759DF,
                            op0=OP.mult, op1=OP.add)
    for _ in range(2):
        nc.vector.tensor_mul(u[:], y[:], y[:])
        nc.vector.tensor_mul(u[:], u[:], var[:])
        nc.vector.tensor_scalar(out=u[:], in0=u[:], scalar1=-0.5, scalar2=1.5,
                                op0=OP.mult, op1=OP.add)
        nc.vector.tensor_mul(y[:], y[:], u[:])
    nc.vector.tensor_copy(out=stats[:, 1:2], in_=y[:])

    # broadcast [mean, rstd] to all 4 channel chunks in one PSUM tile, then
    # compute a = gamma*rstd and b = beta - mean*a for ALL chunks in 3 DVE ops
    h_s = c.hp.tile([P, CC, HW], F16, tag="h")
    mb_p = c.pss.tile([P, CC, 2], F32, tag="small")
    for ci in range(CC):
        nc.tensor.matmul(mb_p[:, ci, :], c.b_sb[:, ci, :], stats[:],
                         start=True, stop=True, skip_group_check=True)
    ab = c.sp.tile([P, CC, 2], F32, tag="ab")       # [:, ci, 0]=a, [:, ci, 1]=b
    nc.vector.tensor_mul(ab[:, :, 0], mb_p[:, :, 1], c.gam_s[:])
    nc.vector.tensor_mul(ab[:, :, 1], mb_p[:, :, 0], ab[:, :, 0])
    nc.vector.tensor_sub(ab[:, :, 1], c.bet_s[:], ab[:, :, 1])
    for ci in range(CC):
        nc.vector.tensor_scalar(out=h_s[:, ci, :], in0=x_s[:, ci, :],
                                scalar1=ab[:, ci, 0:1], scalar2=ab[:, ci, 1:2],
                                op0=OP.mult, op1=OP.add)
        # residual prep: x += bp_eff (bp + Wp @ bv, folded on host)
        nc.vector.tensor_scalar_add(x_s[:, ci, :], x_s[:, ci, :],
                                    c.bpe_s[:, ci:ci + 1])
    return x_s, h_s


def _attn(c, img, xh, prep_next):
    nc = c.nc
    x_s, h_s = xh

    # ---- projections: q, k in (C, HW) layout; v transposed to (HW, C) ----
    q_s = c.qp.tile([P, CC, HW], F16, tag="q")
    k_s = c.kp.tile([P, CC, HW], F16, tag="k")
    for w_s, b_s, o_s in ((c.wq_s, c.bq_s, q_s), (c.wk_s, c.bk_s, k_s)):
        for oc in range(CC):
            for nh in range(NH):
                pq = c.ps.tile([P, 512], F32, tag="mm")
                for ci in range(CC):
                    nc.tensor.matmul(pq[:], w_s[:, ci, ts(oc, P)],
                                     h_s[:, ci, ts(nh, 512)],
                                     start=(ci == 0), stop=(ci == CC - 1))
                nc.scalar.activation(out=o_s[:, oc, ts(nh, 512)], in_=pq[:],
                                     func=AF.Identity, bias=b_s[:, oc:oc + 1])
    vt_s = c.vp.tile([P, MC, C], F16, tag="vt")
    for mc in range(MC):
        pv = c.ps.tile([P, 512], F32, tag="mm")
        for ci in range(CC):
            nc.tensor.matmul(pv[:], h_s[:, ci, ts(mc, P)], c.wv_s[:, ci, :],
                             start=(ci == 0), stop=(ci == CC - 1))
        nc.vector.tensor_copy(out=vt_s[:, mc, :], in_=pv[:])

    # ---- overlap: prep of the next image hides under the attention matmuls
    # (emitted here so its DVE work lands before this image's U copy-outs in
    # the in-order DVE stream, filling DVE idle during the S^T phase)
    nxt = prep_next() if prep_next is not None else None

    # ---- S^T = k^T q (key index m on partitions), exp fused on copy-out ----
    e_s = c.ep.tile([P, MC, HW], F16, tag="exps")
    for mc in range(MC):
        for nh in range(NH):
            px = c.ps.tile([P, 512], F32, tag="mm")
            for ci in range(CC):
                nc.tensor.matmul(px[:], k_s[:, ci, ts(mc, P)],
                                 q_s[:, ci, ts(nh, 512)],
                                 start=(ci == 0), stop=(ci == CC - 1))
            nc.scalar.activation(out=e_s[:, mc, ts(nh, 512)], in_=px[:],
                                 func=AF.Exp, scale=ISC, bias=c.eshift[:])

    # ---- softmax denominators ----
    # sum the 8 key chunks with a 3-deep pairwise tree on DVE (pipelined
    # behind the exps), so the partition reduction needs only ONE ones-matmul
    # per half instead of 8 accumulating ones-matmuls on the PE
    et = c.dp.tile([P, HW], F16, tag="etree")
    esum = c.dp.tile([P, HW], F16, tag="esum")
    nc.vector.tensor_add(esum[:], e_s[:, 0, :], e_s[:, 1, :])
    for j in range(1, 4):
        nc.vector.tensor_add(et[:], e_s[:, 2 * j, :], e_s[:, 2 * j + 1, :])
        nc.vector.tensor_add(esum[:], esum[:], et[:])
    # ---- attention output: U = (v expS^T) * rden, normalized on copy-out ----
    # (the per-pixel softmax denominator commutes with the channel projection).
    # The denominator/broadcast matmuls are emitted AFTER the first AV group:
    # they wait on the DVE esum chain, and the PE queue is in-order, so going
    # first they would stall the whole AV phase behind them.
    u_s = c.up.tile([P, CC, HW], F16, tag="u")
    rden = c.rp.tile([P, HW], F32, tag="rden")
    pu0 = []
    for nh in range(NH):
        pu = c.ps.tile([P, 512], F32, tag="mm", name=f"pu0_{nh}")
        for mc in range(MC):
            nc.tensor.matmul(pu[:], vt_s[:, mc, :P],
                             e_s[:, mc, ts(nh, 512)],
                             start=(mc == 0), stop=(mc == MC - 1))
        pu0.append(pu)
    dens16 = c.sp.tile([1, HW], F16, tag="dens")
    for nh in range(NH):
        dn = c.psd.tile([1, 512], F32, tag="den")
        nc.tensor.matmul(dn[:], c.ones16[:], esum[:, ts(nh, 512)],
                         start=True, stop=True)
        nc.scalar.copy(out=dens16[:, ts(nh, 512)], in_=dn[:])
    for nh in range(NH):
        prb = c.ps.tile([P, 512], F32, tag="mm")
        nc.tensor.matmul(prb[:], c.onesrow[:], dens16[:, ts(nh, 512)],
                         start=True, stop=True)
        nc.vector.reciprocal_approx_fast(out=rden[:, ts(nh, 512)], in_=prb[:])
    for nh in range(NH):
        nc.vector.tensor_mul(u_s[:, 0, ts(nh, 512)], pu0[nh][:],
                             rden[:, ts(nh, 512)])
    for oc in range(1, CC):
        for nh in range(NH):
            pu = c.ps.tile([P, 512], F32, tag="mm")
            for mc in range(MC):
                nc.tensor.matmul(pu[:], vt_s[:, mc, ts(oc, P)],
                                 e_s[:, mc, ts(nh, 512)],
                                 start=(mc == 0), stop=(mc == MC - 1))
            nc.vector.tensor_mul(u_s[:, oc, ts(nh, 512)], pu[:],
                                 rden[:, ts(nh, 512)])

    # ---- output projection + residual; store per channel chunk ----
    for oc2 in range(CC):
        for nh in range(NH):
            pp = c.ps.tile([P, 512], F32, tag="mm")
            for oc in range(CC):
                nc.tensor.matmul(pp[:], c.wp_s[:, oc, ts(oc2, P)],
                                 u_s[:, oc, ts(nh, 512)],
                                 start=(oc == 0), stop=(oc == CC - 1))
            nc.vector.tensor_add(x_s[:, oc2, ts(nh, 512)],
                                 x_s[:, oc2, ts(nh, 512)], pp[:])
            nc.sync.dma_start(out=c.out_d.ap()[img, :, oc2, ts(nh, 512)],
                              in_=x_s[:, oc2, ts(nh, 512)])
    return nxt


def _prep_inputs(x, gn_gamma, gn_beta, wq, bq, wk, bk, wv, bv, wp, bp):
    """Host-side layout shuffles shared by every core."""

    def wt(w):  # (C_out, C_in) -> [p, ci, o] fp16 with contraction dim on partitions
        return np.ascontiguousarray(
            w.T.reshape(CC, P, C).transpose(1, 0, 2)).astype(np.float16)

    def col(v):  # (C,) -> (P, CC) per-partition layout
        return np.ascontiguousarray(v.reshape(CC, P).T).astype(np.float32)

    cidx = np.arange(C)
    gmat = (cidx[:, None] // GSIZE == np.arange(NGROUPS)[None, :]).astype(np.float32)
    gmat = np.ascontiguousarray(gmat.reshape(CC, P, NGROUPS).transpose(1, 0, 2))
    bmat = np.ascontiguousarray(gmat.transpose(2, 1, 0))  # (NGROUPS, CC, P)

    bpe = bp.astype(np.float64) + wp.astype(np.float64) @ bv.astype(np.float64)
    shared = {
        "wqt": wt(wq), "wkt": wt(wk), "wvt": wt(wv), "wpt": wt(wp),
        "bq": col(bq), "bk": col(bk), "bpe": col(bpe.astype(np.float32)),
        "gam": col(gn_gamma), "bet": col(gn_beta),
        "gmat": gmat, "bmat": bmat,
    }
    # x: (B, C, H, W) -> (B, P, CC, HW)
    xr = np.ascontiguousarray(
        np.asarray(x, dtype=np.float32)
        .reshape(B, CC, P, HW).transpose(0, 2, 1, 3))
    return shared, xr


def kernel(x, gn_gamma, gn_beta, wq, bq, wk, bk, wv, bv, wp, bp, _trace=False):
    nc = _build()
    shared, xr = _prep_inputs(np.asarray(x), np.asarray(gn_gamma),
                              np.asarray(gn_beta), np.asarray(wq), np.asarray(bq),
                              np.asarray(wk), np.asarray(bk), np.asarray(wv),
                              np.asarray(bv), np.asarray(wp), np.asarray(bp))
    in_maps = []
    for cix in range(NCORES):
        m = dict(shared)
        m["x"] = np.ascontiguousarray(xr[cix * IPC:(cix + 1) * IPC])
        in_maps.append(m)
    res = run_bass_kernel_spmd(nc, in_maps, core_ids=list(range(NCORES)),
                               trace=_trace)
    if _trace:
        _CACHE["last_result"] = res
    out = np.empty((B, C, H, W), np.float32)
    for cix in range(NCORES):
        o = res.results[cix]["out"]  # (IPC, P, CC, HW)
        out[cix * IPC:(cix + 1) * IPC] = (
            o.transpose(0, 2, 1, 3).reshape(IPC, C, H, W))
    return out


# revision 42
# speedup vs baseline: 1.0320x; 1.0320x over previous
"""AttnBlock++ (GroupNorm + 1x1-conv QKV + full spatial self-attention + proj + residual)
for Trainium2, data-parallel over batch across 8 NeuronCores.

Reference computation (per image, C=512 channels, N=HW=1024 pixels):
  h   = GroupNorm32(x) * gamma + beta
  q,k,v = Wq h + bq, Wk h + bk, Wv h + bv          (1x1 convs = channel matmuls)
  S   = q^T k / sqrt(C);  P = softmax_rows(S);  a = v P^T
  out = x + Wp a + bp

Kernel strategy (per core, 4 images):
  - channels live on partitions in 4 chunks of 128; pixels on the free dim.
  - S is computed TRANSPOSED (key index m on partitions) so the AV contraction
    (over m) is a natural matmul; softmax runs without max subtraction (exp of
    unit-variance scores fits fp32 comfortably); denominators are accumulated
    with a ones-vector matmul and divided out only at the very end (softmax
    normalization commutes with the channel projections).
  - matmul operands are fp16 (PSUM accumulation stays fp32): ~3e-4 max rel
    error per matmul at 4x the fp32 matmul throughput.
  - GroupNorm cross-partition group sums use tiny fp32 matmuls against a
    one-hot group-indicator matrix; rstd is a Quake-style rsqrt (bit-trick
    seed + 2 Newton steps) on the vector engine so the scalar engine never
    leaves the exp activation-table set (a Sqrt/Ln would force a table
    reload per image, ~1.3us each).
  - per-image prep (x load + GroupNorm) of image i+1 is emitted in the middle
    of image i's attention so its DMA/DVE work hides under matmuls.
  - all weight transposes / layout shuffles / bias folds (bv folds into an
    effective bp since softmax rows sum to 1) happen on the host in numpy.
"""

import numpy as np

import concourse.bacc as bacc
import concourse.tile as tile
import concourse.mybir as mybir
from concourse.bass import ts
from concourse.bass_utils import run_bass_kernel_spmd

F32 = mybir.dt.float32
F16 = mybir.dt.float16
AF = mybir.ActivationFunctionType
OP = mybir.AluOpType

B, C, H, W = 32, 512, 32, 32
HW = H * W                    # 1024 pixels
NCORES = 8
IPC = B // NCORES             # images per core
P = 128                       # partitions
CC = C // P                   # 4 channel chunks
MC = HW // P                  # 8 pixel chunks (key index)
NH = HW // 512                # 2 free-dim halves of the pixel axis
NGROUPS = 32
GSIZE = C // NGROUPS          # 16 channels per group
EPS = 1e-5
ISC = float(C) ** -0.5

_CACHE = {}


class _Ctx:
    pass


def _build():
    if "nc" in _CACHE:
        return _CACHE["nc"]
    nc = bacc.Bacc("TRN2", target_bir_lowering=False, debug=False, num_devices=NCORES)

    c = _Ctx()
    c.nc = nc
    c.x_d = nc.dram_tensor("x", (IPC, P, CC, HW), F32, kind="ExternalInput")
    c.wq_d = nc.dram_tensor("wqt", (P, CC, C), F16, kind="ExternalInput")
    c.wk_d = nc.dram_tensor("wkt", (P, CC, C), F16, kind="ExternalInput")
    c.wv_d = nc.dram_tensor("wvt", (P, CC, C), F16, kind="ExternalInput")
    c.wp_d = nc.dram_tensor("wpt", (P, CC, C), F16, kind="ExternalInput")
    c.bq_d = nc.dram_tensor("bq", (P, CC), F32, kind="ExternalInput")
    c.bk_d = nc.dram_tensor("bk", (P, CC), F32, kind="ExternalInput")
    c.bpe_d = nc.dram_tensor("bpe", (P, CC), F32, kind="ExternalInput")
    c.gam_d = nc.dram_tensor("gam", (P, CC), F32, kind="ExternalInput")
    c.bet_d = nc.dram_tensor("bet", (P, CC), F32, kind="ExternalInput")
    c.g_d = nc.dram_tensor("gmat", (P, CC, NGROUPS), F32, kind="ExternalInput")
    c.b_d = nc.dram_tensor("bmat", (NGROUPS, CC, P), F32, kind="ExternalInput")
    c.out_d = nc.dram_tensor("out", (IPC, P, CC, HW), F32, kind="ExternalOutput")

    with tile.TileContext(nc) as tc:
        with (
            tc.tile_pool(name="consts", bufs=1) as cp,
            tc.tile_pool(name="xp", bufs=2) as xp,
            tc.tile_pool(name="hp", bufs=2) as hp,
            tc.tile_pool(name="qp", bufs=2) as qp,
            tc.tile_pool(name="kp", bufs=2) as kp,
            tc.tile_pool(name="vp", bufs=2) as vp,
            tc.tile_pool(name="ep", bufs=2) as ep,
            tc.tile_pool(name="up", bufs=2) as up,
            tc.tile_pool(name="rp", bufs=2) as rp,
            tc.tile_pool(name="smalls", bufs=4) as sp,
            tc.tile_pool(name="dtree", bufs=2) as dp,
            tc.tile_pool(name="ps", bufs=6, space="PSUM") as ps,
            tc.tile_pool(name="pss", bufs=2, space="PSUM") as pss,
        ):
            c.tc, c.cp, c.xp, c.hp, c.qp, c.kp, c.vp, c.ep, c.up = \
                tc, cp, xp, hp, qp, kp, vp, ep, up
            c.rp, c.sp, c.dp, c.ps, c.pss = rp, sp, dp, ps, pss

            c.wq_s = cp.tile([P, CC, C], F16, tag="wq")
            c.wk_s = cp.tile([P, CC, C], F16, tag="wk")
            c.wv_s = cp.tile([P, CC, C], F16, tag="wv")
            c.wp_s = cp.tile([P, CC, C], F16, tag="wp")
            c.bq_s = cp.tile([P, CC], F32, tag="bq")
            c.bk_s = cp.tile([P, CC], F32, tag="bk")
            c.bpe_s = cp.tile([P, CC], F32, tag="bpe")
            c.gam_s = cp.tile([P, CC], F32, tag="gam")
            c.bet_s = cp.tile([P, CC], F32, tag="bet")
            c.g_sb = cp.tile([P, CC, NGROUPS], F32, tag="gmat")
            c.b_sb = cp.tile([NGROUPS, CC, P], F32, tag="bmat")
            c.ones16 = cp.tile([P, 1], F16, tag="ones16")
            c.ones128 = cp.tile([P, P], F16, tag="ones128")
            # constant exp shift: exp(s - 6*ln2) = exp(s)/64. Cancels exactly in
            # the softmax ratio but keeps the fp16 denominator row (a sum of
            # 1024 exps) far from fp16 overflow.
            c.eshift = cp.tile([P, 1], F32, tag="eshift")

            # small consts first so image 0's GroupNorm isn't queued behind
            # the 2 MB of projection weights
            for dst, src in (
                (c.g_sb, c.g_d), (c.b_sb, c.b_d), (c.gam_s, c.gam_d),
                (c.bet_s, c.bet_d), (c.bq_s, c.bq_d), (c.bk_s, c.bk_d),
                (c.bpe_s, c.bpe_d),
            ):
                nc.gpsimd.dma_start(out=dst[:], in_=src.ap())
            nc.vector.memset(c.ones16[:], 1.0)
            nc.vector.memset(c.ones128[:], 1.0)
            nc.vector.memset(c.eshift[:], -6.0 * float(np.log(2.0)))

            # PE warmup: image 0's GroupNorm leaves the PE idle for ~10us at
            # kernel start; junk matmuls in that window ramp the clock gate to
            # full speed before the first real matmul arrives.
            wk_t = c.cp.tile([P, 512], F16, tag="warm")
            nc.vector.memset(wk_t[:], 0.0)
            for i in range(16):
                pw = c.ps.tile([P, 512], F32, tag="mm")
                nc.tensor.matmul(pw[:], wk_t[:, :P], wk_t[:], start=True, stop=True)

            pending = _prep(c, 0)
            for dst, src in (
                (c.wq_s, c.wq_d), (c.wk_s, c.wk_d), (c.wv_s, c.wv_d),
                (c.wp_s, c.wp_d),
            ):
                nc.gpsimd.dma_start(out=dst[:], in_=src.ap())
            for img in range(IPC):
                nxt = _attn(c, img, pending,
                            prep_next=(lambda: _prep(c, img + 1))
                            if img + 1 < IPC else None)
                pending = nxt

    nc.compile()
    _CACHE["nc"] = nc
    return nc


def _prep(c, img):
    """Emit x load + GroupNorm for one image; returns (x_s, h_s)."""
    nc = c.nc
    x_s = c.xp.tile([P, CC, HW], F32, tag="x")
    # per-partition [sum, sumsq] per half-chunk (halves pipeline the DMA)
    scol = c.sp.tile([P, CC, 2, 2], F32, tag="scol")
    junk = c.sp.tile([P, 512], F16, tag="junk")     # discarded Square output
    for ci in range(CC):
        nc.sync.dma_start(out=x_s[:, ci, :], in_=c.x_d.ap()[img, :, ci, :])
        for hf in range(2):
            sl = x_s[:, ci, ts(hf, 512)]
            nc.vector.reduce_sum(out=scol[:, ci, hf, 0:1], in_=sl,
                                 axis=mybir.AxisListType.X)
            nc.scalar.activation(out=junk[:], in_=sl, func=AF.Square,
                                 accum_out=scol[:, ci, hf, 1:2])
    gs_p = c.pss.tile([P, 2], F32, tag="small")     # group [sum, sumsq]
    for i, (ci, hf) in enumerate([(ci, hf) for ci in range(CC) for hf in range(2)]):
        nc.tensor.matmul(gs_p[:NGROUPS, :], c.g_sb[:, ci, :], scol[:, ci, hf, :],
                         start=(i == 0), stop=(i == 2 * CC - 1))
    inv_n = 1.0 / float(GSIZE * HW)
    mv = c.sp.tile([NGROUPS, 2], F32, tag="mv")     # [mean, E(x^2)]
    nc.vector.tensor_scalar_mul(mv[:], gs_p[:NGROUPS, :], inv_n)
    stats = c.sp.tile([NGROUPS, 2], F32, tag="stats")  # [mean, rstd]
    nc.vector.tensor_copy(out=stats[:, 0:1], in_=mv[:, 0:1])
    var = c.sp.tile([NGROUPS, 1], F32, tag="var")
    nc.vector.tensor_mul(var[:], mv[:, 0:1], mv[:, 0:1])
    nc.vector.tensor_sub(var[:], mv[:, 1:2], var[:])
    # rstd = 1/sqrt(var+eps) via Quake seed + 2 Newton steps, all on DVE.
    # (a Sqrt/Ln on ACT would force an activation-table reload per image; the
    # exp set stays resident this way)
    I32 = mybir.dt.int32
    nc.vector.tensor_scalar_add(var[:], var[:], EPS)
    y = c.sp.tile([NGROUPS, 1], F32, tag="rsq_y")
    u = c.sp.tile([NGROUPS, 1], F32, tag="rsq_u")
    nc.vector.tensor_scalar(out=y[:].bitcast(I32), in0=var[:].bitcast(I32),
                            scalar1=1, scalar2=None,
                            op0=OP.logical_shift_right)
    nc.vector.tensor_scalar(out=y[:].bitcast(I32), in0=y[:].bitcast(I32),
                            scalar1=-1, scalar2=0x5F3759DF,
                            op0=OP.mult, op1=OP.add)
    for _ in range(2):
        nc.vector.tensor_mul(u[:], y[:], y[:])
        nc.vector.tensor_mul(u[:], u[:], var[:])
        nc.vector.tensor_scalar(out=u[:], in0=u[:], scalar1=-0.5, scalar2=1.5,
                                op0=OP.mult, op1=OP.add)
        nc.vector.tensor_mul(y[:], y[:], u[:])
    nc.vector.tensor_copy(out=stats[:, 1:2], in_=y[:])

    # broadcast [mean, rstd] to all 4 channel chunks in one PSUM tile, then
    # compute a = gamma*rstd and b = beta - mean*a for ALL chunks in 3 DVE ops
    h_s = c.hp.tile([P, CC, HW], F16, tag="h")
    mb_p = c.pss.tile([P, CC, 2], F32, tag="small")
    for ci in range(CC):
        nc.tensor.matmul(mb_p[:, ci, :], c.b_sb[:, ci, :], stats[:],
                         start=True, stop=True, skip_group_check=True)
    ab = c.sp.tile([P, CC, 2], F32, tag="ab")       # [:, ci, 0]=a, [:, ci, 1]=b
    nc.vector.tensor_mul(ab[:, :, 0], mb_p[:, :, 1], c.gam_s[:])
    nc.vector.tensor_mul(ab[:, :, 1], mb_p[:, :, 0], ab[:, :, 0])
    nc.vector.tensor_sub(ab[:, :, 1], c.bet_s[:], ab[:, :, 1])
    for ci in range(CC):
        nc.vector.tensor_scalar(out=h_s[:, ci, :], in0=x_s[:, ci, :],
                                scalar1=ab[:, ci, 0:1], scalar2=ab[:, ci, 1:2],
                                op0=OP.mult, op1=OP.add)
        # residual prep: x += bp_eff (bp + Wp @ bv, folded on host)
        nc.vector.tensor_scalar_add(x_s[:, ci, :], x_s[:, ci, :],
                                    c.bpe_s[:, ci:ci + 1])
    return x_s, h_s


def _attn(c, img, xh, prep_next):
    nc = c.nc
    x_s, h_s = xh

    # ---- projections: q, k in (C, HW) layout; v transposed to (HW, C) ----
    q_s = c.qp.tile([P, CC, HW], F16, tag="q")
    k_s = c.kp.tile([P, CC, HW], F16, tag="k")
    for w_s, b_s, o_s in ((c.wq_s, c.bq_s, q_s), (c.wk_s, c.bk_s, k_s)):
        for oc in range(CC):
            for nh in range(NH):
                pq = c.ps.tile([P, 512], F32, tag="mm")
                for ci in range(CC):
                    nc.tensor.matmul(pq[:], w_s[:, ci, ts(oc, P)],
                                     h_s[:, ci, ts(nh, 512)],
                                     start=(ci == 0), stop=(ci == CC - 1))
                nc.scalar.activation(out=o_s[:, oc, ts(nh, 512)], in_=pq[:],
                                     func=AF.Identity, bias=b_s[:, oc:oc + 1])
    vt_s = c.vp.tile([P, MC, C], F16, tag="vt")
    for mc in range(MC):
        pv = c.ps.tile([P, 512], F32, tag="mm")
        for ci in range(CC):
            nc.tensor.matmul(pv[:], h_s[:, ci, ts(mc, P)], c.wv_s[:, ci, :],
                             start=(ci == 0), stop=(ci == CC - 1))
        nc.vector.tensor_copy(out=vt_s[:, mc, :], in_=pv[:])

    # ---- overlap: prep of the next image hides under the attention matmuls
    # (emitted here so its DVE work lands before this image's U copy-outs in
    # the in-order DVE stream, filling DVE idle during the S^T phase)
    nxt = prep_next() if prep_next is not None else None

    # ---- S^T = k^T q (key index m on partitions), exp fused on copy-out ----
    e_s = c.ep.tile([P, MC, HW], F16, tag="exps")
    for mc in range(MC):
        for nh in range(NH):
            px = c.ps.tile([P, 512], F32, tag="mm")
            for ci in range(CC):
                nc.tensor.matmul(px[:], k_s[:, ci, ts(mc, P)],
                                 q_s[:, ci, ts(nh, 512)],
                                 start=(ci == 0), stop=(ci == CC - 1))
            nc.scalar.activation(out=e_s[:, mc, ts(nh, 512)], in_=px[:],
                                 func=AF.Exp, scale=ISC, bias=c.eshift[:])

    # ---- softmax denominators ----
    # sum the 8 key chunks with a 3-deep pairwise tree on DVE (pipelined
    # behind the exps), so the partition reduction needs only ONE ones-matmul
    # per half instead of 8 accumulating ones-matmuls on the PE
    et = c.dp.tile([P, HW], F16, tag="etree")
    esum = c.dp.tile([P, HW], F16, tag="esum")
    nc.vector.tensor_add(esum[:], e_s[:, 0, :], e_s[:, 1, :])
    for j in range(1, 4):
        nc.vector.tensor_add(et[:], e_s[:, 2 * j, :], e_s[:, 2 * j + 1, :])
        nc.vector.tensor_add(esum[:], esum[:], et[:])
    # ---- attention output: U = (v expS^T) * rden, normalized on copy-out ----
    # (the per-pixel softmax denominator commutes with the channel projection).
    # The denominator/broadcast matmuls are emitted AFTER the first AV group:
    # they wait on the DVE esum chain, and the PE queue is in-order, so going
    # first they would stall the whole AV phase behind them.
    u_s = c.up.tile([P, CC, HW], F16, tag="u")
    rden = c.rp.tile([P, HW], F32, tag="rden")
    pu0 = []
    for nh in range(NH):
        pu = c.ps.tile([P, 512], F32, tag="mm", name=f"pu0_{nh}")
        for mc in range(MC):
            nc.tensor.matmul(pu[:], vt_s[:, mc, :P],
                             e_s[:, mc, ts(nh, 512)],
                             start=(mc == 0), stop=(mc == MC - 1))
        pu0.append(pu)
    # one all-ones stationary matmul per half fuses the partition reduction
    # AND the broadcast: out[p, n] = sum_k esum[k, n] for every p
    for nh in range(NH):
        prb = c.ps.tile([P, 512], F32, tag="mm")
        nc.tensor.matmul(prb[:], c.ones128[:], esum[:, ts(nh, 512)],
                         start=True, stop=True)
        nc.vector.reciprocal_approx_fast(out=rden[:, ts(nh, 512)], in_=prb[:])
    for nh in range(NH):
        nc.vector.tensor_mul(u_s[:, 0, ts(nh, 512)], pu0[nh][:],
                             rden[:, ts(nh, 512)])
    for oc in range(1, CC):
        for nh in range(NH):
            pu = c.ps.tile([P, 512], F32, tag="mm")
            for mc in range(MC):
                nc.tensor.matmul(pu[:], vt_s[:, mc, ts(oc, P)],
                                 e_s[:, mc, ts(nh, 512)],
                                 start=(mc == 0), stop=(mc == MC - 1))
            nc.vector.tensor_mul(u_s[:, oc, ts(nh, 512)], pu[:],
                                 rden[:, ts(nh, 512)])

    # ---- output projection + residual; store per channel chunk ----
    for oc2 in range(CC):
        for nh in range(NH):
            pp = c.ps.tile([P, 512], F32, tag="mm")
            for oc in range(CC):
                nc.tensor.matmul(pp[:], c.wp_s[:, oc, ts(oc2, P)],
                                 u_s[:, oc, ts(nh, 512)],
                                 start=(oc == 0), stop=(oc == CC - 1))
            nc.vector.tensor_add(x_s[:, oc2, ts(nh, 512)],
                                 x_s[:, oc2, ts(nh, 512)], pp[:])
            nc.sync.dma_start(out=c.out_d.ap()[img, :, oc2, ts(nh, 512)],
                              in_=x_s[:, oc2, ts(nh, 512)])
    return nxt


def _prep_inputs(x, gn_gamma, gn_beta, wq, bq, wk, bk, wv, bv, wp, bp):
    """Host-side layout shuffles shared by every core."""

    def wt(w):  # (C_out, C_in) -> [p, ci, o] fp16 with contraction dim on partitions
        return np.ascontiguousarray(
            w.T.reshape(CC, P, C).transpose(1, 0, 2)).astype(np.float16)

    def col(v):  # (C,) -> (P, CC) per-partition layout
        return np.ascontiguousarray(v.reshape(CC, P).T).astype(np.float32)

    cidx = np.arange(C)
    gmat = (cidx[:, None] // GSIZE == np.arange(NGROUPS)[None, :]).astype(np.float32)
    gmat = np.ascontiguousarray(gmat.reshape(CC, P, NGROUPS).transpose(1, 0, 2))
    bmat = np.ascontiguousarray(gmat.transpose(2, 1, 0))  # (NGROUPS, CC, P)

    bpe = bp.astype(np.float64) + wp.astype(np.float64) @ bv.astype(np.float64)
    shared = {
        "wqt": wt(wq), "wkt": wt(wk), "wvt": wt(wv), "wpt": wt(wp),
        "bq": col(bq), "bk": col(bk), "bpe": col(bpe.astype(np.float32)),
        "gam": col(gn_gamma), "bet": col(gn_beta),
        "gmat": gmat, "bmat": bmat,
    }
    # x: (B, C, H, W) -> (B, P, CC, HW)
    xr = np.ascontiguousarray(
        np.asarray(x, dtype=np.float32)
        .reshape(B, CC, P, HW).transpose(0, 2, 1, 3))
    return shared, xr


def kernel(x, gn_gamma, gn_beta, wq, bq, wk, bk, wv, bv, wp, bp, _trace=False):
    nc = _build()
    shared, xr = _prep_inputs(np.asarray(x), np.asarray(gn_gamma),
                              np.asarray(gn_beta), np.asarray(wq), np.asarray(bq),
                              np.asarray(wk), np.asarray(bk), np.asarray(wv),
                              np.asarray(bv), np.asarray(wp), np.asarray(bp))
    in_maps = []
    for cix in range(NCORES):
        m = dict(shared)
        m["x"] = np.ascontiguousarray(xr[cix * IPC:(cix + 1) * IPC])
        in_maps.append(m)
    res = run_bass_kernel_spmd(nc, in_maps, core_ids=list(range(NCORES)),
                               trace=_trace)
    if _trace:
        _CACHE["last_result"] = res
    out = np.empty((B, C, H, W), np.float32)
    for cix in range(NCORES):
        o = res.results[cix]["out"]  # (IPC, P, CC, HW)
        out[cix * IPC:(cix + 1) * IPC] = (
            o.transpose(0, 2, 1, 3).reshape(IPC, C, H, W))
    return out


# revision 43
# speedup vs baseline: 1.0324x; 1.0003x over previous
"""AttnBlock++ (GroupNorm + 1x1-conv QKV + full spatial self-attention + proj + residual)
for Trainium2, data-parallel over batch across 8 NeuronCores.

Reference computation (per image, C=512 channels, N=HW=1024 pixels):
  h   = GroupNorm32(x) * gamma + beta
  q,k,v = Wq h + bq, Wk h + bk, Wv h + bv          (1x1 convs = channel matmuls)
  S   = q^T k / sqrt(C);  P = softmax_rows(S);  a = v P^T
  out = x + Wp a + bp

Kernel strategy (per core, 4 images):
  - channels live on partitions in 4 chunks of 128; pixels on the free dim.
  - S is computed TRANSPOSED (key index m on partitions) so the AV contraction
    (over m) is a natural matmul; softmax runs without max subtraction (exp of
    unit-variance scores fits fp32 comfortably); denominators are accumulated
    with a ones-vector matmul and divided out only at the very end (softmax
    normalization commutes with the channel projections).
  - matmul operands are fp16 (PSUM accumulation stays fp32): ~3e-4 max rel
    error per matmul at 4x the fp32 matmul throughput.
  - GroupNorm cross-partition group sums use tiny fp32 matmuls against a
    one-hot group-indicator matrix; rstd is a Quake-style rsqrt (bit-trick
    seed + 2 Newton steps) on the vector engine so the scalar engine never
    leaves the exp activation-table set (a Sqrt/Ln would force a table
    reload per image, ~1.3us each).
  - per-image prep (x load + GroupNorm) of image i+1 is emitted in the middle
    of image i's attention so its DMA/DVE work hides under matmuls.
  - all weight transposes / layout shuffles / bias folds (bv folds into an
    effective bp since softmax rows sum to 1) happen on the host in numpy.
"""

import numpy as np

import concourse.bacc as bacc
import concourse.tile as tile
import concourse.mybir as mybir
from concourse.bass import ts
from concourse.bass_utils import run_bass_kernel_spmd

F32 = mybir.dt.float32
F16 = mybir.dt.float16
AF = mybir.ActivationFunctionType
OP = mybir.AluOpType

B, C, H, W = 32, 512, 32, 32
HW = H * W                    # 1024 pixels
NCORES = 8
IPC = B // NCORES             # images per core
P = 128                       # partitions
CC = C // P                   # 4 channel chunks
MC = HW // P                  # 8 pixel chunks (key index)
NH = HW // 512                # 2 free-dim halves of the pixel axis
NGROUPS = 32
GSIZE = C // NGROUPS          # 16 channels per group
EPS = 1e-5
ISC = float(C) ** -0.5

_CACHE = {}


class _Ctx:
    pass


def _build():
    if "nc" in _CACHE:
        return _CACHE["nc"]
    nc = bacc.Bacc("TRN2", target_bir_lowering=False, debug=False, num_devices=NCORES)

    c = _Ctx()
    c.nc = nc
    c.x_d = nc.dram_tensor("x", (IPC, P, CC, HW), F32, kind="ExternalInput")
    c.wq_d = nc.dram_tensor("wqt", (P, CC, C), F16, kind="ExternalInput")
    c.wk_d = nc.dram_tensor("wkt", (P, CC, C), F16, kind="ExternalInput")
    c.wv_d = nc.dram_tensor("wvt", (P, CC, C), F16, kind="ExternalInput")
    c.wp_d = nc.dram_tensor("wpt", (P, CC, C), F16, kind="ExternalInput")
    c.bq_d = nc.dram_tensor("bq", (P, CC), F32, kind="ExternalInput")
    c.bk_d = nc.dram_tensor("bk", (P, CC), F32, kind="ExternalInput")
    c.bpe_d = nc.dram_tensor("bpe", (P, CC), F32, kind="ExternalInput")
    c.gam_d = nc.dram_tensor("gam", (P, CC), F32, kind="ExternalInput")
    c.bet_d = nc.dram_tensor("bet", (P, CC), F32, kind="ExternalInput")
    c.g_d = nc.dram_tensor("gmat", (P, CC, NGROUPS), F32, kind="ExternalInput")
    c.b_d = nc.dram_tensor("bmat", (NGROUPS, CC, P), F32, kind="ExternalInput")
    c.out_d = nc.dram_tensor("out", (IPC, P, CC, HW), F32, kind="ExternalOutput")

    with tile.TileContext(nc) as tc:
        with (
            tc.tile_pool(name="consts", bufs=1) as cp,
            tc.tile_pool(name="xp", bufs=2) as xp,
            tc.tile_pool(name="hp", bufs=2) as hp,
            tc.tile_pool(name="qp", bufs=2) as qp,
            tc.tile_pool(name="kp", bufs=2) as kp,
            tc.tile_pool(name="vp", bufs=2) as vp,
            tc.tile_pool(name="ep", bufs=2) as ep,
            tc.tile_pool(name="up", bufs=2) as up,
            tc.tile_pool(name="rp", bufs=2) as rp,
            tc.tile_pool(name="smalls", bufs=4) as sp,
            tc.tile_pool(name="dtree", bufs=2) as dp,
            tc.tile_pool(name="ps", bufs=6, space="PSUM") as ps,
            tc.tile_pool(name="pss", bufs=2, space="PSUM") as pss,
        ):
            c.tc, c.cp, c.xp, c.hp, c.qp, c.kp, c.vp, c.ep, c.up = \
                tc, cp, xp, hp, qp, kp, vp, ep, up
            c.rp, c.sp, c.dp, c.ps, c.pss = rp, sp, dp, ps, pss

            c.wq_s = cp.tile([P, CC, C], F16, tag="wq")
            c.wk_s = cp.tile([P, CC, C], F16, tag="wk")
            c.wv_s = cp.tile([P, CC, C], F16, tag="wv")
            c.wp_s = cp.tile([P, CC, C], F16, tag="wp")
            c.bq_s = cp.tile([P, CC], F32, tag="bq")
            c.bk_s = cp.tile([P, CC], F32, tag="bk")
            c.bpe_s = cp.tile([P, CC], F32, tag="bpe")
            c.gam_s = cp.tile([P, CC], F32, tag="gam")
            c.bet_s = cp.tile([P, CC], F32, tag="bet")
            c.g_sb = cp.tile([P, CC, NGROUPS], F32, tag="gmat")
            c.b_sb = cp.tile([NGROUPS, CC, P], F32, tag="bmat")
            c.ones16 = cp.tile([P, 1], F16, tag="ones16")
            c.ones128 = cp.tile([P, P], F16, tag="ones128")
            # constant exp shift: exp(s - 6*ln2) = exp(s)/64. Cancels exactly in
            # the softmax ratio but keeps the fp16 denominator row (a sum of
            # 1024 exps) far from fp16 overflow.
            c.eshift = cp.tile([P, 1], F32, tag="eshift")

            # small consts first so image 0's GroupNorm isn't queued behind
            # the 2 MB of projection weights
            for dst, src in (
                (c.g_sb, c.g_d), (c.b_sb, c.b_d), (c.gam_s, c.gam_d),
                (c.bet_s, c.bet_d), (c.bq_s, c.bq_d), (c.bk_s, c.bk_d),
                (c.bpe_s, c.bpe_d),
            ):
                nc.gpsimd.dma_start(out=dst[:], in_=src.ap())
            nc.vector.memset(c.ones16[:], 1.0)
            nc.vector.memset(c.ones128[:], 1.0)
            nc.vector.memset(c.eshift[:], -6.0 * float(np.log(2.0)))

            # PE warmup: image 0's GroupNorm leaves the PE idle for ~10us at
            # kernel start; junk matmuls in that window ramp the clock gate to
            # full speed before the first real matmul arrives.
            wk_t = c.cp.tile([P, 512], F16, tag="warm")
            nc.vector.memset(wk_t[:], 0.0)
            for i in range(16):
                pw = c.ps.tile([P, 512], F32, tag="mm")
                nc.tensor.matmul(pw[:], wk_t[:, :P], wk_t[:], start=True, stop=True)

            pending = _prep(c, 0)
            for dst, src in (
                (c.wq_s, c.wq_d), (c.wk_s, c.wk_d), (c.wv_s, c.wv_d),
                (c.wp_s, c.wp_d),
            ):
                nc.gpsimd.dma_start(out=dst[:], in_=src.ap())
            for img in range(IPC):
                nxt = _attn(c, img, pending,
                            prep_next=(lambda: _prep(c, img + 1))
                            if img + 1 < IPC else None)
                pending = nxt

    nc.compile()
    _CACHE["nc"] = nc
    return nc


def _prep(c, img):
    """Emit x load + GroupNorm for one image; returns (x_s, h_s)."""
    nc = c.nc
    x_s = c.xp.tile([P, CC, HW], F32, tag="x")
    # per-partition [sum, sumsq] per half-chunk (halves pipeline the DMA)
    scol = c.sp.tile([P, CC, 2, 2], F32, tag="scol")
    junk = c.sp.tile([P, 512], F16, tag="junk")     # discarded Square output
    for ci in range(CC):
        nc.sync.dma_start(out=x_s[:, ci, :], in_=c.x_d.ap()[img, :, ci, :])
        for hf in range(2):
            sl = x_s[:, ci, ts(hf, 512)]
            nc.vector.reduce_sum(out=scol[:, ci, hf, 0:1], in_=sl,
                                 axis=mybir.AxisListType.X)
            nc.scalar.activation(out=junk[:], in_=sl, func=AF.Square,
                                 accum_out=scol[:, ci, hf, 1:2])
    gs_p = c.pss.tile([P, 2], F32, tag="small")     # group [sum, sumsq]
    for i, (ci, hf) in enumerate([(ci, hf) for ci in range(CC) for hf in range(2)]):
        nc.tensor.matmul(gs_p[:NGROUPS, :], c.g_sb[:, ci, :], scol[:, ci, hf, :],
                         start=(i == 0), stop=(i == 2 * CC - 1))
    stats = c.sp.tile([NGROUPS, 2], F32, tag="stats")  # [mean, rstd]
    nc.vector.tensor_copy(out=stats[:, 0:1], in_=gs_p[:NGROUPS, 0:1])
    var = c.sp.tile([NGROUPS, 1], F32, tag="var")
    nc.vector.tensor_mul(var[:], stats[:, 0:1], stats[:, 0:1])
    nc.vector.tensor_sub(var[:], gs_p[:NGROUPS, 1:2], var[:])
    # rstd = 1/sqrt(var+eps) via Quake seed + 2 Newton steps, all on DVE.
    # (a Sqrt/Ln on ACT would force an activation-table reload per image; the
    # exp set stays resident this way)
    I32 = mybir.dt.int32
    nc.vector.tensor_scalar_add(var[:], var[:], EPS)
    y = c.sp.tile([NGROUPS, 1], F32, tag="rsq_y")
    u = c.sp.tile([NGROUPS, 1], F32, tag="rsq_u")
    nc.vector.tensor_scalar(out=y[:].bitcast(I32), in0=var[:].bitcast(I32),
                            scalar1=1, scalar2=None,
                            op0=OP.logical_shift_right)
    nc.vector.tensor_scalar(out=y[:].bitcast(I32), in0=y[:].bitcast(I32),
                            scalar1=-1, scalar2=0x5F3759DF,
                            op0=OP.mult, op1=OP.add)
    for _ in range(2):
        nc.vector.tensor_mul(u[:], y[:], y[:])
        nc.vector.tensor_mul(u[:], u[:], var[:])
        nc.vector.tensor_scalar(out=u[:], in0=u[:], scalar1=-0.5, scalar2=1.5,
                                op0=OP.mult, op1=OP.add)
        nc.vector.tensor_mul(y[:], y[:], u[:])
    nc.vector.tensor_copy(out=stats[:, 1:2], in_=y[:])

    # broadcast [mean, rstd] to all 4 channel chunks in one PSUM tile, then
    # compute a = gamma*rstd and b = beta - mean*a for ALL chunks in 3 DVE ops
    h_s = c.hp.tile([P, CC, HW], F16, tag="h")
    mb_p = c.pss.tile([P, CC, 2], F32, tag="small")
    for ci in range(CC):
        nc.tensor.matmul(mb_p[:, ci, :], c.b_sb[:, ci, :], stats[:],
                         start=True, stop=True, skip_group_check=True)
    ab = c.sp.tile([P, CC, 2], F32, tag="ab")       # [:, ci, 0]=a, [:, ci, 1]=b
    nc.vector.tensor_mul(ab[:, :, 0], mb_p[:, :, 1], c.gam_s[:])
    nc.vector.tensor_mul(ab[:, :, 1], mb_p[:, :, 0], ab[:, :, 0])
    nc.vector.tensor_sub(ab[:, :, 1], c.bet_s[:], ab[:, :, 1])
    for ci in range(CC):
        nc.vector.tensor_scalar(out=h_s[:, ci, :], in0=x_s[:, ci, :],
                                scalar1=ab[:, ci, 0:1], scalar2=ab[:, ci, 1:2],
                                op0=OP.mult, op1=OP.add)
        # residual prep: x += bp_eff (bp + Wp @ bv, folded on host)
        nc.vector.tensor_scalar_add(x_s[:, ci, :], x_s[:, ci, :],
                                    c.bpe_s[:, ci:ci + 1])
    return x_s, h_s


def _attn(c, img, xh, prep_next):
    nc = c.nc
    x_s, h_s = xh

    # ---- projections: q, k in (C, HW) layout; v transposed to (HW, C) ----
    q_s = c.qp.tile([P, CC, HW], F16, tag="q")
    k_s = c.kp.tile([P, CC, HW], F16, tag="k")
    for w_s, b_s, o_s in ((c.wq_s, c.bq_s, q_s), (c.wk_s, c.bk_s, k_s)):
        for oc in range(CC):
            for nh in range(NH):
                pq = c.ps.tile([P, 512], F32, tag="mm")
                for ci in range(CC):
                    nc.tensor.matmul(pq[:], w_s[:, ci, ts(oc, P)],
                                     h_s[:, ci, ts(nh, 512)],
                                     start=(ci == 0), stop=(ci == CC - 1))
                nc.scalar.activation(out=o_s[:, oc, ts(nh, 512)], in_=pq[:],
                                     func=AF.Identity, bias=b_s[:, oc:oc + 1])
    vt_s = c.vp.tile([P, MC, C], F16, tag="vt")
    for mc in range(MC):
        pv = c.ps.tile([P, 512], F32, tag="mm")
        for ci in range(CC):
            nc.tensor.matmul(pv[:], h_s[:, ci, ts(mc, P)], c.wv_s[:, ci, :],
                             start=(ci == 0), stop=(ci == CC - 1))
        nc.vector.tensor_copy(out=vt_s[:, mc, :], in_=pv[:])

    # ---- overlap: prep of the next image hides under the attention matmuls
    # (emitted here so its DVE work lands before this image's U copy-outs in
    # the in-order DVE stream, filling DVE idle during the S^T phase)
    nxt = prep_next() if prep_next is not None else None

    # ---- S^T = k^T q (key index m on partitions), exp fused on copy-out ----
    e_s = c.ep.tile([P, MC, HW], F16, tag="exps")
    for mc in range(MC):
        for nh in range(NH):
            px = c.ps.tile([P, 512], F32, tag="mm")
            for ci in range(CC):
                nc.tensor.matmul(px[:], k_s[:, ci, ts(mc, P)],
                                 q_s[:, ci, ts(nh, 512)],
                                 start=(ci == 0), stop=(ci == CC - 1))
            nc.scalar.activation(out=e_s[:, mc, ts(nh, 512)], in_=px[:],
                                 func=AF.Exp, scale=ISC, bias=c.eshift[:])

    # ---- softmax denominators ----
    # sum the 8 key chunks with a 3-deep pairwise tree on DVE (pipelined
    # behind the exps), so the partition reduction needs only ONE ones-matmul
    # per half instead of 8 accumulating ones-matmuls on the PE
    et = c.dp.tile([P, HW], F16, tag="etree")
    esum = c.dp.tile([P, HW], F16, tag="esum")
    nc.vector.tensor_add(esum[:], e_s[:, 0, :], e_s[:, 1, :])
    for j in range(1, 4):
        nc.vector.tensor_add(et[:], e_s[:, 2 * j, :], e_s[:, 2 * j + 1, :])
        nc.vector.tensor_add(esum[:], esum[:], et[:])
    # ---- attention output: U = (v expS^T) * rden, normalized on copy-out ----
    # (the per-pixel softmax denominator commutes with the channel projection).
    # The denominator/broadcast matmuls are emitted AFTER the first AV group:
    # they wait on the DVE esum chain, and the PE queue is in-order, so going
    # first they would stall the whole AV phase behind them.
    u_s = c.up.tile([P, CC, HW], F16, tag="u")
    rden = c.rp.tile([P, HW], F32, tag="rden")
    pu0 = []
    for nh in range(NH):
        pu = c.ps.tile([P, 512], F32, tag="mm", name=f"pu0_{nh}")
        for mc in range(MC):
            nc.tensor.matmul(pu[:], vt_s[:, mc, :P],
                             e_s[:, mc, ts(nh, 512)],
                             start=(mc == 0), stop=(mc == MC - 1))
        pu0.append(pu)
    # one all-ones stationary matmul per half fuses the partition reduction
    # AND the broadcast: out[p, n] = sum_k esum[k, n] for every p
    for nh in range(NH):
        prb = c.ps.tile([P, 512], F32, tag="mm")
        nc.tensor.matmul(prb[:], c.ones128[:], esum[:, ts(nh, 512)],
                         start=True, stop=True)
        nc.vector.reciprocal_approx_fast(out=rden[:, ts(nh, 512)], in_=prb[:])
    for nh in range(NH):
        nc.vector.tensor_mul(u_s[:, 0, ts(nh, 512)], pu0[nh][:],
                             rden[:, ts(nh, 512)])
    for oc in range(1, CC):
        for nh in range(NH):
            pu = c.ps.tile([P, 512], F32, tag="mm")
            for mc in range(MC):
                nc.tensor.matmul(pu[:], vt_s[:, mc, ts(oc, P)],
                                 e_s[:, mc, ts(nh, 512)],
                                 start=(mc == 0), stop=(mc == MC - 1))
            nc.vector.tensor_mul(u_s[:, oc, ts(nh, 512)], pu[:],
                                 rden[:, ts(nh, 512)])

    # ---- output projection + residual; store per channel chunk ----
    for oc2 in range(CC):
        for nh in range(NH):
            pp = c.ps.tile([P, 512], F32, tag="mm")
            for oc in range(CC):
                nc.tensor.matmul(pp[:], c.wp_s[:, oc, ts(oc2, P)],
                                 u_s[:, oc, ts(nh, 512)],
                                 start=(oc == 0), stop=(oc == CC - 1))
            nc.vector.tensor_add(x_s[:, oc2, ts(nh, 512)],
                                 x_s[:, oc2, ts(nh, 512)], pp[:])
            nc.sync.dma_start(out=c.out_d.ap()[img, :, oc2, ts(nh, 512)],
                              in_=x_s[:, oc2, ts(nh, 512)])
    return nxt


def _prep_inputs(x, gn_gamma, gn_beta, wq, bq, wk, bk, wv, bv, wp, bp):
    """Host-side layout shuffles shared by every core."""

    def wt(w):  # (C_out, C_in) -> [p, ci, o] fp16 with contraction dim on partitions
        return np.ascontiguousarray(
            w.T.reshape(CC, P, C).transpose(1, 0, 2)).astype(np.float16)

    def col(v):  # (C,) -> (P, CC) per-partition layout
        return np.ascontiguousarray(v.reshape(CC, P).T).astype(np.float32)

    cidx = np.arange(C)
    gmat = (cidx[:, None] // GSIZE == np.arange(NGROUPS)[None, :]).astype(np.float32)
    gmat = np.ascontiguousarray(gmat.reshape(CC, P, NGROUPS).transpose(1, 0, 2))
    bmat = np.ascontiguousarray(gmat.transpose(2, 1, 0))  # (NGROUPS, CC, P)
    # fold the 1/(group size) normalization into the summing matrix so the
    # group-sum matmul directly yields [mean, E(x^2)]
    gmat = gmat * np.float32(1.0 / (GSIZE * HW))

    bpe = bp.astype(np.float64) + wp.astype(np.float64) @ bv.astype(np.float64)
    shared = {
        "wqt": wt(wq), "wkt": wt(wk), "wvt": wt(wv), "wpt": wt(wp),
        "bq": col(bq), "bk": col(bk), "bpe": col(bpe.astype(np.float32)),
        "gam": col(gn_gamma), "bet": col(gn_beta),
        "gmat": gmat, "bmat": bmat,
    }
    # x: (B, C, H, W) -> (B, P, CC, HW)
    xr = np.ascontiguousarray(
        np.asarray(x, dtype=np.float32)
        .reshape(B, CC, P, HW).transpose(0, 2, 1, 3))
    return shared, xr


def kernel(x, gn_gamma, gn_beta, wq, bq, wk, bk, wv, bv, wp, bp, _trace=False):
    nc = _build()
    shared, xr = _prep_inputs(np.asarray(x), np.asarray(gn_gamma),
                              np.asarray(gn_beta), np.asarray(wq), np.asarray(bq),
                              np.asarray(wk), np.asarray(bk), np.asarray(wv),
                              np.asarray(bv), np.asarray(wp), np.asarray(bp))
    in_maps = []
    for cix in range(NCORES):
        m = dict(shared)
        m["x"] = np.ascontiguousarray(xr[cix * IPC:(cix + 1) * IPC])
        in_maps.append(m)
    res = run_bass_kernel_spmd(nc, in_maps, core_ids=list(range(NCORES)),
                               trace=_trace)
    if _trace:
        _CACHE["last_result"] = res
    out = np.empty((B, C, H, W), np.float32)
    for cix in range(NCORES):
        o = res.results[cix]["out"]  # (IPC, P, CC, HW)
        out[cix * IPC:(cix + 1) * IPC] = (
            o.transpose(0, 2, 1, 3).reshape(IPC, C, H, W))
    return out
